# revision 37
# baseline (speedup 1.0000x reference)
"""Trainium2 Bass kernel for nn_MetaMultiHeadSelfAttention_45810121179385.

Multi-head causal self-attention: B=4, S=2048, D=1024, H=16 heads (hd=64).

Sharding (8 NeuronCores): batch (4) x head-group (2 groups of 8 heads).
Core c handles batch b = c//2, head group g = c%2:
  - QKV projections for its 512 head-dims (tensor parallel on d_k rows)
  - attention for its 8 heads (full sequence, causal)
  - partial o_proj (columns of o_proj for its 512 v-dims)
Host sums the two partial outputs per batch and stacks batches.

All matmul operands are bf16 (1 cyc/row at any output width on TRN2's PE;
accumulation stays fp32 in PSUM).  Attention structure per head:
  - scores^T [kpos-tile 128, q window] pieces in PSUM -> exp on Act -> e_t
    (bf16 SBUF), causal mask = one 128x128 multiply on the diagonal tile.
  - PV uses e_t as lhsT: out[q-tile 128, 65] = e_t_tile^T @ V65 where V65
    carries a ones column, so each matmul streams only 65 columns and the
    softmax denominator lands per-q-partition in column 64.  Normalize is a
    per-partition reciprocal + tensor_scalar multiply (no broadcasts).
  - normalized [q, v] tiles go back to [v, q] via XBAR DMA transposes.
The attention loop is q-tile-outer with all 16 k-tiles' exp tensors retained
compactly in SBUF (34.8KB/partition), soeach q-tile's PV accumulation is
one consecutive run in its own PSUM bank (PSUM allows only one open
accumulation group per bank).  QKV projections, per-pair V chunks, and the
first 3/4 of o_proj (into a bf16 y_half, folded back later via DVE adds and
identity-matmul injection) are interleaved into the emission as per-iteration
fill, sized to each head's Act-over-PE deficit, so the PE stays dense while
Act streams exps.  Next k-tile scores are emitted last in each iteration so
a stalled PSUM ring slot never blocks ready work in the PE's in-order queue.

PSUM budget (8 banks x 2KB): scores ring 2 slots x 2 banks, PV 2 slots x 1
bank, projection/fill 2 slots x 1 bank.
"""

import functools
import os
import sys
from contextlib import ExitStack

import numpy as np

sys.path.insert(0, "/opt/trn_rl_repo")

import ml_dtypes  # noqa: E402

import concourse.bass as bass  # noqa: E402
import concourse.tile as tile  # noqa: E402
from concourse import bacc, mybir  # noqa: E402
from concourse.bass_utils import run_bass_kernel_spmd  # noqa: E402

F32 = mybir.dt.float32
BF16 = mybir.dt.bfloat16
EXP = mybir.ActivationFunctionType.Exp
BF = ml_dtypes.bfloat16

B, S, D, H, HD = 4, 2048, 1024, 16, 64
NCORES = 8
HPC = 8          # heads per core
GD = HPC * HD    # 512 head-dims per core
NKT = S // 128   # 16 kpos tiles of 128
NQT = S // 128   # 16 q tiles of 128
NDC = D // 128   # 8 contraction chunks for projections
NVT = GD // 128  # 4 dk/v tiles per core
SCALE = 1.0 / np.sqrt(HD)
PIECE = 1024     # scores piece width (2 PSUM banks)

DEFAULT_OPTS = {}
OPTS = dict(DEFAULT_OPTS)
VARIANTS = {"": {}, "x2": {"nbody": 2}, "x4": {"nbody": 4}, "x8": {"nbody": 8}}


class _Ctx:
    """Bundles the resident tiles + pools used across emission helpers."""
    pass


def _emit_qk_block(c, t, sc, which):
    """One Q^T or K^T projection block: dk-tile t, s-chunk sc."""
    nc = c.nc
    w_sb, dst = (c.wq_sb, c.qt_sb) if which == 0 else (c.wk_sb, c.kt_sb)
    ps = c.proj.tile([128, 512], F32, tag="proj", name=f"psqk{t}_{sc}_{which}")
    for k in range(NDC):
        nc.tensor.matmul(
            ps,
            lhsT=w_sb[:, k, 128 * t : 128 * (t + 1)],
            rhs=c.x_sb[:, k, 512 * sc : 512 * (sc + 1)],
            start=(k == 0),
            stop=(k == NDC - 1),
        )
    nc.vector.tensor_copy(out=dst[:, t, 512 * sc : 512 * (sc + 1)], in_=ps)


def _emit_v_chunk(c, kti, p):
    """V projection for s-tile kti, head pair p only ([128, 128] columns).
    Splitting by pair defers most v work to just before its consumer pair."""
    nc = c.nc
    ps = c.proj.tile([128, 128], F32, tag="proj", name=f"psv{kti}_{p}")
    for k in range(NDC):
        nc.tensor.matmul(
            ps,
            lhsT=c.x_sb[:, k, 128 * kti : 128 * (kti + 1)],
            rhs=c.wv_sb[:, k, 128 * p : 128 * (p + 1)],
            start=(k == 0),
            stop=(k == NDC - 1),
        )
    nc.vector.tensor_copy(
        out=c.v_sb[:, kti, 2 * p : 2 * p + 2, 0:HD],
        in_=ps.rearrange("p (h d) -> p h d", h=2),
    )


def _emit_oproj_half1(c, m, qc):
    """First-half o_proj partial (v-tiles 0..2) -> y_half bf16."""
    nc = c.nc
    ps = c.proj.tile([128, 512], F32, tag="proj", name=f"psh1_{m}_{qc}")
    for t in range(3):
        nc.tensor.matmul(
            ps,
            lhsT=c.wo_sb[:, t, 128 * m : 128 * (m + 1)],
            rhs=c.ot_sbT[:, t, 512 * qc : 512 * (qc + 1)],
            start=(t == 0),
            stop=(t == 2),
        )
    nc.vector.tensor_copy(out=c.yh_sb[:, m, 512 * qc : 512 * (qc + 1)], in_=ps)


def _emit_scores(c, h, kt):
    """scores^T [kpos, q] for k-tile kt -> exp -> e_t; mask on diagonal.
    e_t is stored compactly: its column 0 is global q column 128*kt."""
    nc = c.nc
    t_h, p_h = h // 2, 64 * (h % 2)
    c_lo = 128 * kt
    W = S - c_lo
    e_t = c.epool.tile([128, W], BF16, tag=f"e{kt}", name=f"e{h}_{kt}")
    pieces = [(c_lo, min(PIECE, W))]
    if W > PIECE:
        pieces.append((c_lo + PIECE, W - PIECE))
    for pi, (a, w) in enumerate(pieces):
        sc_ps = c.ring.tile([128, PIECE], F32, tag="ring", name=f"sc{h}_{kt}")
        col = 0
        while col < w:
            ncols = min(512, w - col)
            nc.tensor.matmul(
                sc_ps[:, col : col + ncols],
                lhsT=c.kt_sb[p_h : p_h + 64, t_h, c_lo : c_lo + 128],
                rhs=c.qt_sb[p_h : p_h + 64, t_h, a + col : a + col + ncols],
                start=True,
                stop=True,
            )
            col += ncols
        nc.scalar.activation(
            out=e_t[:, a - c_lo : a - c_lo + w], in_=sc_ps[:, 0:w],
            func=EXP, scale=SCALE,
        )
        if pi == 0:
            # causal mask on the diagonal 128x128 block
            nc.vector.tensor_mul(e_t[:, 0:128], e_t[:, 0:128], c.mask_sb)
    return e_t


def _emit_attn_head(c, h, hook, pre_hook=None):
    """Full attention for head h, q-tile-outer: each q-tile's PV accumulation
    is a consecutive run of matmuls into its own PSUM bank (PSUM allows only
    one open accumulation group per bank).  Exp tiles for all k-tiles are
    retained compactly in SBUF for the whole head.  hook(i) emits interleaved
    work (projection chunks) after iteration i; pre_hook() is emitted right
    after the first scores block so its PE work overlaps the first exp."""
    nc = c.nc
    t_h, p_h = h // 2, 64 * (h % 2)
    e_tiles = [None] * NKT
    e_tiles[0] = _emit_scores(c, h, 0)
    if pre_hook is not None:
        pre_hook()
    for qt in range(NQT):
        pv = c.pvp.tile([128, HD + 1], F32, tag="pv", name=f"pv{h}_{qt}")
        # old k-tiles first (their exps are long done); the diagonal k-tile
        # last, overlapping Act's fresh exp with the streaming of old tiles
        for kt in range(qt + 1):
            e_t = e_tiles[kt]
            nc.tensor.matmul(
                pv,
                lhsT=e_t[:, 128 * (qt - kt) : 128 * (qt - kt + 1)],
                rhs=c.v_sb[:, kt, h, :],
                start=(kt == 0),
                stop=(kt == qt),
            )
        rec = c.rpool.tile([128, 1], F32, tag="rec", name=f"rec{h}_{qt}")
        nc.vector.reciprocal(out=rec, in_=pv[:, HD : HD + 1])
        nc.vector.tensor_scalar_mul(
            c.ot_qv[:, qt, p_h : p_h + HD], pv[:, 0:HD], rec
        )
        if h == 0 and f"e{qt}" in c.dbg:
            nc.sync.dma_start(out=c.dbg[f"e{qt}"], in_=e_tiles[qt])
        hook(qt)
        # next k-tile's scores go last: if their PSUM ring slot is still
        # held by a pending exp, the ready work above isn't stuck behind
        # them in the PE's in-order queue
        if qt + 1 < NKT:
            e_tiles[qt + 1] = _emit_scores(c, h, qt + 1)


def _emit_qt_transpose(c, t, qt):
    """XBAR DMA transpose of one normalized [q, v] tile into ot_sbT [v, q]
    (SBUF->SBUF, bf16) — no PE or DVE involvement."""
    c.nc.sync.dma_start_transpose(
        out=c.ot_sbT[:, t, 128 * qt : 128 * (qt + 1)],
        in_=c.ot_qv[:, qt, :],
    )


def _mha_tile_kernel(tc, xT, wqT, wkT, wvT, woT, mask, ident, yT, dbg=None):
    nc = tc.nc
    c = _Ctx()
    c.nc = nc
    c.dbg = dbg or {}

    es = ExitStack()
    with es:
        qkv = es.enter_context(tc.tile_pool(name="qkv", bufs=1))
        opool = es.enter_context(tc.tile_pool(name="outT", bufs=1))
        cpool = es.enter_context(tc.tile_pool(name="consts", bufs=1))
        wopool = es.enter_context(tc.tile_pool(name="wop", bufs=1))
        c.wo_sb = wopool.tile([128, NVT, D], BF16, tag="wo")
        c.qt_sb = qkv.tile([128, NVT, S], BF16, tag="qt")
        c.kt_sb = qkv.tile([128, NVT, S], BF16, tag="kt")
        c.v_sb = qkv.tile([128, NKT, HPC, HD + 1], BF16, tag="v")
        c.ot_sbT = opool.tile([128, NVT, S], BF16, tag="ot")
        c.mask_sb = cpool.tile([128, 128], BF16, tag="mask")
        c.ident_sb = cpool.tile([128, 128], BF16, tag="ident")

        xT_r = xT.rearrange("(k p) s -> p k s", p=128)
        wqT_r = wqT.rearrange("(k p) g -> p k g", p=128)
        wkT_r = wkT.rearrange("(k p) g -> p k g", p=128)
        wvT_r = wvT.rearrange("(k p) g -> p k g", p=128)

        attn_es = ExitStack()
        with attn_es:
            ring = attn_es.enter_context(
                tc.tile_pool(name="ring", bufs=2, space="PSUM"))
            pvp = attn_es.enter_context(
                tc.tile_pool(name="pvp", bufs=2, space="PSUM"))
            projp = attn_es.enter_context(
                tc.tile_pool(name="projp", bufs=2, space="PSUM"))
            epool = attn_es.enter_context(tc.tile_pool(name="ep", bufs=1))
            oqvpool = attn_es.enter_context(tc.tile_pool(name="oqv", bufs=2))
            rpool = attn_es.enter_context(tc.tile_pool(name="rp", bufs=4))
            c.ring, c.pvp, c.epool, c.rpool = ring, pvp, epool, rpool
            c.proj = projp

            # phase-1 residents (x + qkv weights) live only until the last
            # projection block; their SBUF is then reused for y_half
            ph1 = ExitStack()
            xpool = ph1.enter_context(tc.tile_pool(name="xp", bufs=1))
            wpool = ph1.enter_context(tc.tile_pool(name="wqkv", bufs=1))
            c.x_sb = xpool.tile([128, NDC, S], BF16, tag="x")
            c.wq_sb = wpool.tile([128, NDC, GD], BF16, tag="wq")
            c.wk_sb = wpool.tile([128, NDC, GD], BF16, tag="wk")
            c.wv_sb = wpool.tile([128, NDC, GD], BF16, tag="wv")

            # ---- staged input DMAs on three parallel HWDGE queues (SP for
            # wq, Act for x, DVE for wk/wv) so the first q/k blocks' operands
            # land as early as possible ----
            for lo, hi in ((0, 2), (2, 4), (4, 8)):
                nc.sync.dma_start(out=c.wq_sb[:, lo:hi], in_=wqT_r[:, lo:hi])
                nc.sync.dma_start(
                    out=c.x_sb[:, lo:hi, 0:512], in_=xT_r[:, lo:hi, 0:512]
                )
                nc.sync.dma_start(out=c.wk_sb[:, lo:hi], in_=wkT_r[:, lo:hi])
            nc.sync.dma_start(
                out=c.x_sb[:, :, 512:1024], in_=xT_r[:, :, 512:1024]
            )
            nc.sync.dma_start(out=c.wv_sb, in_=wvT_r)
            nc.sync.dma_start(
                out=c.x_sb[:, :, 1024:1536], in_=xT_r[:, :, 1024:1536]
            )
            nc.sync.dma_start(out=c.mask_sb, in_=mask)
            nc.sync.dma_start(out=c.ident_sb, in_=ident)
            nc.sync.dma_start(
                out=c.x_sb[:, :, 1536:2048], in_=xT_r[:, :, 1536:2048]
            )
            nc.sync.dma_start(
                out=c.wo_sb, in_=woT.rearrange("(t p) m -> p t m", p=128)
            )
            nc.gpsimd.memset(c.v_sb[:, :, :, HD : HD + 1], 1.0)

            # ---- projections for pair 0 up front; everything else is
            # interleaved into the attention emission so the PE has work
            # while Act streams exps ----
            for sc in range(4):
                _emit_qk_block(c, 0, sc, 0)
                _emit_qk_block(c, 0, sc, 1)

            def qk_item(t, sc, which):
                return lambda: _emit_qk_block(c, t, sc, which)

            def h1_item(m, qc):
                return lambda: _emit_oproj_half1(c, m, qc)

            # per-(head, iteration) interleave schedule.  Even head 2p fills
            # itself with pair p's v chunks just-in-time (chunk kti lands at
            # iteration kti-2, two ahead of its consumer); odd heads carry
            # the next pairs' q/k projections; h6/h7 carry the o_proj first
            # half.  Each even head's v work (~6.8us) matches its Act-over-PE
            # deficit, so the PE stays dense through the whole attention.
            sched = {}
            for p in range(4):
                for kti in range(2, NKT):
                    sched.setdefault((2 * p, kti - 2), []).append(
                        (lambda kti=kti, p=p: _emit_v_chunk(c, kti, p))
                    )
            qk1 = [qk_item(1, sc, w) for sc in range(4) for w in (0, 1)]
            qk2 = [qk_item(2, sc, w) for sc in range(4) for w in (0, 1)]
            qk3 = [qk_item(3, sc, w) for sc in range(4) for w in (0, 1)]
            for i in range(8):
                sched.setdefault((1, 2 * i), []).append(qk1[i])
                sched.setdefault((3, 2 * i), []).append(qk2[i])
                sched.setdefault((5, 2 * i), []).append(qk3[i])
            for i in range(16):
                sched.setdefault((6, i), []).append(h1_item(i // 4, i % 4))
                sched.setdefault((7, i), []).append(h1_item(4 + i // 4, i % 4))

            def hook_for(h):
                def hook(kt):
                    if h % 2 == 1:
                        # both heads of pair h//2 have normalized q-tile kt
                        _emit_qt_transpose(c, h // 2, kt)
                    for item in sched.get((h, kt), ()):
                        item()
                return hook

            ot_qv_pair = [None] * 4
            for pair in range(4):
                c.ot_qv = ot_qv_pair[pair] = oqvpool.tile(
                    [128, NQT, 128], BF16, tag="oqv", name=f"oqv{pair}"
                )
                def pre(pair=pair):
                    _emit_v_chunk(c, 0, pair)
                    _emit_v_chunk(c, 1, pair)

                if pair == 3:
                    # x/w no longer needed; their SBUF is freed and y_half
                    # goes on the opposite allocator side so it can outlive
                    # the attention pools
                    ph1.close()
                    yhpool = es.enter_context(
                        tc.tile_pool(name="yh", bufs=1, side="right"))
                    c.yh_sb = yhpool.tile([128, D // 128, S], BF16, tag="yh")
                _emit_attn_head(c, 2 * pair, hook_for(2 * pair), pre)
                _emit_attn_head(c, 2 * pair + 1, hook_for(2 * pair + 1))
                if pair == 0 and "ot_qv0" in c.dbg:
                    nc.sync.dma_start(out=c.dbg["ot_qv0"], in_=c.ot_qv)
            for nm, sb in (("qt", c.qt_sb), ("kt", c.kt_sb), ("v", c.v_sb),
                           ("ot", c.ot_sbT)):
                if nm in c.dbg:
                    nc.sync.dma_start(out=c.dbg[nm], in_=sb)
            yh_keep = c.yh_sb

        # ---- tail: last o_proj tile + y_half injected via identity matmul
        with (
            tc.tile_pool(name="ps_o", bufs=6, space="PSUM") as ps_o,
            tc.tile_pool(name="ysb", bufs=3) as ypool,
        ):
            for m in range(D // 128):
                y_sb = ypool.tile([128, S], BF16, tag="y", name=f"y{m}")
                for qc in range(4):
                    ps_y = ps_o.tile([128, 512], F32, tag="psy", name=f"psy{m}_{qc}")
                    if qc % 2 == 0:
                        # DVE folds in the y_half partial during the copy-out
                        nc.tensor.matmul(
                            ps_y,
                            lhsT=c.wo_sb[:, 3, 128 * m : 128 * (m + 1)],
                            rhs=c.ot_sbT[:, 3, 512 * qc : 512 * (qc + 1)],
                            start=True,
                            stop=True,
                        )
                        nc.vector.tensor_add(
                            y_sb[:, 512 * qc : 512 * (qc + 1)],
                            ps_y,
                            yh_keep[:, m, 512 * qc : 512 * (qc + 1)],
                        )
                    else:
                        # Act can't add tensors: inject y_half via an
                        # identity matmul into the accumulation instead
                        nc.tensor.matmul(
                            ps_y,
                            lhsT=c.wo_sb[:, 3, 128 * m : 128 * (m + 1)],
                            rhs=c.ot_sbT[:, 3, 512 * qc : 512 * (qc + 1)],
                            start=True,
                            stop=False,
                        )
                        nc.tensor.matmul(
                            ps_y,
                            lhsT=c.ident_sb,
                            rhs=yh_keep[:, m, 512 * qc : 512 * (qc + 1)],
                            start=False,
                            stop=True,
                        )
                        nc.scalar.copy(
                            out=y_sb[:, 512 * qc : 512 * (qc + 1)], in_=ps_y
                        )
                    if qc % 2 == 1:
                        nc.sync.dma_start(
                            out=yT[
                                128 * m : 128 * (m + 1),
                                1024 * (qc // 2) : 1024 * (qc // 2 + 1),
                            ],
                            in_=y_sb[:, 1024 * (qc // 2) : 1024 * (qc // 2 + 1)],
                        )


@functools.lru_cache(maxsize=8)
def build_program(variant=None):
    if variant is None:
        variant = os.environ.get("MHA_VARIANT", "")
    OPTS.clear()
    OPTS.update(DEFAULT_OPTS)
    OPTS.update(VARIANTS[variant])
    nc = bacc.Bacc("TRN2", target_bir_lowering=False, debug=False)
    xT = nc.dram_tensor("xT", [D, S], BF16, kind="ExternalInput").ap()
    wqT = nc.dram_tensor("wqT", [D, GD], BF16, kind="ExternalInput").ap()
    wkT = nc.dram_tensor("wkT", [D, GD], BF16, kind="ExternalInput").ap()
    wvT = nc.dram_tensor("wvT", [D, GD], BF16, kind="ExternalInput").ap()
    woT = nc.dram_tensor("woT", [GD, D], BF16, kind="ExternalInput").ap()
    mask = nc.dram_tensor("mask", [128, 128], BF16, kind="ExternalInput").ap()
    ident = nc.dram_tensor("ident", [128, 128], BF16, kind="ExternalInput").ap()
    yT = nc.dram_tensor("yT", [D, S], BF16, kind="ExternalOutput").ap()
    for _ in range(OPTS.get("nbody", 1)):
        with tile.TileContext(nc) as tc:
            _mha_tile_kernel(tc, xT, wqT, wkT, wvT, woT, mask, ident, yT)
    nc.compile()
    return nc


def make_in_maps(x, q_proj, k_proj, v_proj, o_proj):
    x = np.asarray(x, dtype=np.float32)
    mask = np.triu(np.ones((128, 128), dtype=BF))  # keep iff col >= row
    ident = np.eye(128, dtype=BF)
    in_maps = []
    for c in range(NCORES):
        b, g = divmod(c, 2)
        sl = slice(GD * g, GD * (g + 1))
        in_maps.append(
            {
                "xT": np.ascontiguousarray(x[b].T).astype(BF),
                "wqT": np.ascontiguousarray(np.asarray(q_proj)[sl, :].T).astype(BF),
                "wkT": np.ascontiguousarray(np.asarray(k_proj)[sl, :].T).astype(BF),
                "wvT": np.ascontiguousarray(np.asarray(v_proj)[sl, :].T).astype(BF),
                "woT": np.ascontiguousarray(np.asarray(o_proj)[:, sl].T).astype(BF),
                "mask": mask,
                "ident": ident,
            }
        )
    return in_maps


def gather_output(results):
    outs = [np.asarray(r["yT"]).astype(np.float32) for r in results]
    return np.stack(
        [(outs[2 * b] + outs[2 * b + 1]).T for b in range(B)], axis=0
    )


def kernel(x, q_proj, k_proj, v_proj, o_proj, _trace=False, _trace_kwargs=None):
    nc = build_program()
    in_maps = make_in_maps(x, q_proj, k_proj, v_proj, o_proj)
    res = run_bass_kernel_spmd(
        nc,
        in_maps,
        core_ids=list(range(NCORES)),
        trace=_trace,
        **(_trace_kwargs or {}),
    )
    y = gather_output(res.results)
    if _trace:
        kernel.last_result = res
    return y
